# revision 12
# baseline (speedup 1.0000x reference)
"""Trainium2 Bass kernel: per-feature 9-layer tiny-MLP CDF model (DistributionFreeModel).

Math per (batch b, feature f), scalar x = inputs[b, f]:
    h = softplus(W0[f]) * x + b0[f]                  (1 -> 3)
    for l in 1..7:  u = softplus(Wl[f]) @ h + bl[f]  (3 -> 3)
    skip (layers 0..7): h = u + tanh(sl[f]) * tanh(u)
    out = sigmoid(softplus(W8[f]) @ h + b8[f])       (3 -> 1)

Device mapping (per core, pure data parallel over batch):
  - Features on SBUF partitions, batch on the free dim. The host pre-transposes
    each core's input slice to feature-major [512, 4096] and re-transposes the
    feature-major output afterwards (pure layout work).
  - The per-feature 3x3 matvecs run on the TensorEngine as block-diagonal
    float32r matmuls (full-rate streaming): 40 features/block -> stationary
    [121, 120] = 3x3 diagonal blocks + a bias row driven by a persistent
    ones-row in the moving tile. All stationaries ship in ONE packed DMA.
  - tanh/sigmoid on ScalarE; the skip combine (t*s + u) is one VectorE
    scalar_tensor_tensor with per-partition s reading u straight from PSUM.
  - Blocks are emitted in interleaved waves of 4 (= PSUM u-tile slots) so each
    engine round-robins across 4 independent 9-layer chains - without this the
    serial MM->tanh->combine chain leaves every engine ~30% utilized.
All parameter preprocessing (softplus/tanh/block-diag packing) happens on the
host in numpy - it is O(F*P), negligible next to the O(B*F) main work.
"""

import sys
import numpy as np
from contextlib import ExitStack

sys.path.insert(0, "/opt/trn_rl_repo")

from concourse import bacc, mybir, tile  # noqa: E402
from concourse.bass_utils import run_bass_kernel_spmd  # noqa: E402
from concourse.mybir import ActivationFunctionType as AF, AluOpType as ALU  # noqa: E402

F32 = mybir.dt.float32
F32R = mybir.dt.float32r
NCORES = 8
B, F, P = 32768, 512, 118
BSH = B // NCORES            # 4096 batch rows per core
BT = 1024                    # batch columns per on-device chunk
SIZES = [40] * 12 + [32]     # features per block-diagonal group (sum = 512)
STARTS = [sum(SIZES[:j]) for j in range(len(SIZES))]
NBLK = len(SIZES)
BLK_STRIDE = 1000            # packed-stats columns per block: 8*120 + 40
WAVE = 4                     # block-chains in flight (= PSUM u slots)
FP32_LAYERS = 5              # layers 0..4 exact fp32; 5..8 float32r (full-rate PE)


def _softplus(x):
    return np.log1p(np.exp(-np.abs(x))) + np.maximum(x, 0.0)


def build_consts(params: np.ndarray) -> dict:
    """Pack softplus'd weights into one padded block-diagonal stats blob.

    Column layout per block j (base = j*BLK_STRIDE):
      [base      : base+120)  layer 0 stationary  [g+1, 3g] (rows 0..g)
      [base+120l : +120)      layer l (1..7)      [3g+1, 3g]
      [base+960  : base+1000) layer 8 stationary  [3g, g]
    """
    params = np.asarray(params, np.float32)
    Ws, Bs, Ss = [], [], []
    Ws.append(_softplus(params[:, 0:3]).reshape(F, 3, 1))
    Bs.append(params[:, 3:6])
    Ss.append(np.tanh(params[:, 6:9]))
    o = 3
    for _l in range(1, 8):
        Ws.append(_softplus(params[:, 3 * o:3 * o + 9]).reshape(F, 3, 3))
        Bs.append(params[:, 3 * o + 9:3 * o + 12])
        Ss.append(np.tanh(params[:, 3 * o + 12:3 * o + 15]))
        o += 5
    Ws.append(_softplus(params[:, 114:117]).reshape(F, 1, 3))
    Bs.append(params[:, 117:118])

    stats = np.zeros((121, NBLK * BLK_STRIDE), np.float32)
    svec = np.zeros((120, 8 * NBLK), np.float32)       # skip scales, col = l*NBLK+j
    b8m = np.zeros((40, NBLK), np.float32)             # final bias, col = j
    for j, (f0, g) in enumerate(zip(STARTS, SIZES)):
        base = j * BLK_STRIDE
        for i in range(g):
            f = f0 + i
            stats[i, base + 3 * i:base + 3 * i + 3] = Ws[0][f, :, 0]
            stats[g, base + 3 * i:base + 3 * i + 3] = Bs[0][f]
            for l in range(1, 8):
                # stat[3i+di, 3i+do] = W[do, di]
                cb = base + 120 * l
                stats[3 * i:3 * i + 3, cb + 3 * i:cb + 3 * i + 3] = Ws[l][f].T
                stats[3 * g, cb + 3 * i:cb + 3 * i + 3] = Bs[l][f]
            stats[3 * i:3 * i + 3, base + 960 + i] = Ws[8][f, 0, :]
            b8m[i, j] = Bs[8][f, 0]
            for l in range(8):
                svec[3 * i:3 * i + 3, l * NBLK + j] = Ss[l][f]
    return dict(
        stats=stats, svec=svec, b8m=b8m,
        ones=np.ones((1, BT), np.float32),
    )


def build_nc(bsh: int = BSH, bt: int = BT, ra=10, rb=4, r0a=6, r0b=2,
             tbufs=6, sigbufs=4, ubufs=WAVE):
    nch = bsh // bt
    nhalf = bt // 512
    nc = bacc.Bacc(None, target_bir_lowering=False)

    xT = nc.dram_tensor("xT", [F, bsh], F32, kind="ExternalInput")
    dStats = nc.dram_tensor("stats", [121, NBLK * BLK_STRIDE], F32, kind="ExternalInput")
    dS = nc.dram_tensor("svec", [120, 8 * NBLK], F32, kind="ExternalInput")
    db8 = nc.dram_tensor("b8m", [40, NBLK], F32, kind="ExternalInput")
    dOne = nc.dram_tensor("ones", [1, bt], F32, kind="ExternalInput")
    yT = nc.dram_tensor("yT", [F, bsh], F32, kind="ExternalOutput")

    with ExitStack() as ctx:
        tc = ctx.enter_context(tile.TileContext(nc))
        cpool = ctx.enter_context(tc.tile_pool(name="const", bufs=1))
        tp = ctx.enter_context(tc.tile_pool(name="tp", bufs=tbufs))
        sgp = ctx.enter_context(tc.tile_pool(name="sgp", bufs=sigbufs))
        pup = ctx.enter_context(tc.tile_pool(name="pup", bufs=ubufs, space="PSUM"))

        stats = cpool.tile([121, NBLK * BLK_STRIDE], F32, tag="stats")
        nc.sync.dma_start(stats[:].bitcast(F32R), dStats[:].bitcast(F32R))
        sv = cpool.tile([120, 8 * NBLK], F32, tag="sv")
        nc.sync.dma_start(sv[:], dS[:])
        b8t = cpool.tile([40, NBLK], F32, tag="b8")
        nc.sync.dma_start(b8t[:], db8[:])

        def statA(j, g):
            b = j * BLK_STRIDE
            return stats[0:g + 1, b:b + 3 * g]

        def statB(l, j, g):  # l in 1..7
            b = j * BLK_STRIDE + 120 * l
            ap = stats[0:3 * g + 1, b:b + 3 * g]
            return ap.bitcast(F32R) if l >= FP32_LAYERS else ap

        def statC(j, g):
            b = j * BLK_STRIDE + 960
            return stats[0:3 * g, b:b + g].bitcast(F32R)

        # Moving-operand rings with a persistent ones row at partition 3g / g.
        # Separate rings per matmul precision: tiles consumed by f32r matmuls
        # must only ever be written with f32r-declared outputs (BIR verifier
        # tracks producers per tile, across ring reuse).
        mvFA = [cpool.tile([121, bt], F32, tag=f"mvFA{r}", name=f"mvFA{r}") for r in range(6)]
        mvFB = [cpool.tile([97, bt], F32, tag=f"mvFB{r}", name=f"mvFB{r}") for r in range(2)]
        mvRA = [cpool.tile([121, bt], F32, tag=f"mvRA{r}", name=f"mvRA{r}") for r in range(6)]
        mvRB = [cpool.tile([97, bt], F32, tag=f"mvRB{r}", name=f"mvRB{r}") for r in range(2)]
        m0A = [cpool.tile([41, bt], F32, tag=f"m0A{r}", name=f"m0A{r}") for r in range(r0a)]
        m0B = [cpool.tile([33, bt], F32, tag=f"m0B{r}", name=f"m0B{r}") for r in range(r0b)]
        for t_ in mvFA:
            nc.sync.dma_start(t_[120:121, :], dOne[:])
        for t_ in mvFB:
            nc.sync.dma_start(t_[96:97, :], dOne[:])
        for t_ in mvRA:
            nc.sync.dma_start(t_[120:121, :].bitcast(F32R), dOne[:].bitcast(F32R))
        for t_ in mvRB:
            nc.sync.dma_start(t_[96:97, :].bitcast(F32R), dOne[:].bitcast(F32R))
        for t_ in m0A:
            nc.sync.dma_start(t_[40:41, :], dOne[:])
        for t_ in m0B:
            nc.sync.dma_start(t_[32:33, :], dOne[:])
        rix = {}

        def _next(ring, key):
            i = rix.get(key, 0)
            rix[key] = i + 1
            return ring[i % len(ring)]

        def next_mv(g, rounded):
            if rounded:
                return _next(mvRA if g == 40 else mvRB, f"R{g}")
            return _next(mvFA if g == 40 else mvFB, f"F{g}")

        def next_m0(g):
            return _next(m0A if g == 40 else m0B, f"0{g}")

        def emit_skip_and_next(blk, l):
            """Skip combine for layer l, then the layer l+1 matmuls."""
            g, g3, u = blk["g"], 3 * blk["g"], blk["u"]
            t_ = tp.tile([120, bt], F32, tag="t", name="t")
            nc.scalar.activation(t_[0:g3, :], u[0:g3, :], AF.Tanh)
            rounded = (l + 1) >= FP32_LAYERS
            nxt = next_mv(g, rounded)
            col = l * NBLK + blk["j"]
            out_ap = nxt[0:g3, :]
            if rounded:
                out_ap = out_ap.bitcast(F32R)
            nc.vector.scalar_tensor_tensor(
                out_ap, t_[0:g3, :], sv[0:g3, col:col + 1], u[0:g3, :],
                ALU.mult, ALU.add,
            )
            for h in range(nhalf):
                hs = slice(h * 512, (h + 1) * 512)
                if l < 7:
                    mv_ap = nxt[0:g3 + 1, hs]
                    if rounded:
                        mv_ap = mv_ap.bitcast(F32R)
                    nc.tensor.matmul(
                        u[0:g3, hs], statB(l + 1, blk["j"], g), mv_ap,
                        start=True, stop=True,
                    )
                else:
                    nc.tensor.matmul(
                        u[0:g, hs], statC(blk["j"], g),
                        nxt[0:g3, hs].bitcast(F32R), start=True, stop=True,
                    )

        def enter(c, j):
            f0, g = STARTS[j], SIZES[j]
            mv0 = next_m0(g)
            nc.sync.dma_start(mv0[0:g, :], xT[f0:f0 + g, c * bt:(c + 1) * bt])
            u = pup.tile([120, bt], F32, tag="u", name="u")
            for h in range(nhalf):
                hs = slice(h * 512, (h + 1) * 512)
                nc.tensor.matmul(
                    u[0:3 * g, hs], statA(j, g), mv0[0:g + 1, hs],
                    start=True, stop=True,
                )
            return dict(c=c, j=j, f0=f0, g=g, u=u, layer=0)

        def retire(blk):
            c, j, f0, g, u = blk["c"], blk["j"], blk["f0"], blk["g"], blk["u"]
            sig = sgp.tile([40, bt], F32, tag="sig", name="sig")
            nc.scalar.activation(
                sig[0:g, :], u[0:g, :], AF.Sigmoid, bias=b8t[0:g, j:j + 1]
            )
            nc.sync.dma_start(yT[f0:f0 + g, c * bt:(c + 1) * bt], sig[0:g, :])

        # Staggered software pipeline: at most one block enters per step, so
        # the WAVE in-flight blocks sit at staggered layers and every step
        # mixes fp32- and f32r-layer work across PE/ACT/DVE.
        from collections import deque
        pending = deque((c, j) for c in range(nch) for j in range(NBLK))
        active = deque()
        while pending or active:
            if len(active) < WAVE and pending:
                active.append(enter(*pending.popleft()))
            done = []
            for blk in active:
                emit_skip_and_next(blk, blk["layer"])
                blk["layer"] += 1
                if blk["layer"] == 8:
                    done.append(blk)
            for blk in done:
                retire(blk)
                active.remove(blk)

    nc.compile()
    return nc


_NC_CACHE = {}


def kernel(inputs: np.ndarray, parameters: np.ndarray) -> np.ndarray:
    inputs = np.asarray(inputs, np.float32)
    consts = build_consts(parameters)
    if "hw" not in _NC_CACHE:
        _NC_CACHE["hw"] = build_nc(BSH, BT)
    nc = _NC_CACHE["hw"]
    in_maps = []
    for c in range(NCORES):
        m = dict(consts)
        m["xT"] = np.ascontiguousarray(inputs[c * BSH:(c + 1) * BSH, :].T)
        in_maps.append(m)
    res = run_bass_kernel_spmd(nc, in_maps, list(range(NCORES))).results
    out = np.empty((B, F), np.float32)
    for c in range(NCORES):
        out[c * BSH:(c + 1) * BSH, :] = res[c]["yT"].T
    return out
